# revision 11
# baseline (speedup 1.0000x reference)
"""Multi-head causal self-attention (B=2, S=2048, D=1024, H=16) on 8 TRN2 cores.

Sharding: head-parallel attention + token-parallel output projection.
Core c owns head-group c = heads {2c, 2c+1} (128 of the 1024 qkv dims,
both batches) for projections+attention, then tokens [hf*1024 + c*128)
of each batch for the output projection (Wo replicated).

v2 schedule (HAM-aware: keep ONE dense PE stream so the engine stays at
warm clock; the v1 baseline ran 85% of the kernel at 1.2 GHz):

  B(b0):  Q/K/V^T projections for batch 0, N=1024 matmuls, 3-PSUM.
  C(b0):  V^T -> V_aug [tok, 65] tiles (col 64 = ones -> l row trick).
  D(b0) interleaved with B(b1)+C prep: per (qc, kt) the ACT exp paces the
          pipeline; 2-3 B(b1) projection matmuls slot into each PE gap.
  E(b,hf) batched per half-batch: one Ln+Exp on [1, 2*1024] (not per-qc),
          r broadcast via PE outer product, normalize to bf16, stage the
          AllToAll send buffer. A2A(b,hf0) fires mid-D(b), (b,hf1) at end.
  C(b1) burst, then D(b1) with gx prefetches on sync.
  F: transposed output projection: stationary = resharded ctx^T chunk
     [128 dims, 128 tok], moving = wo rows [128 dims, 1024 od], N=1024.
     8 matmuls + 1 DVE bias-add per (b,hf); F(0,*),F(1,0) fill the last
     A2A's flight. Output y[tok, od] bf16 (host casts to f32).

Host: x pre-transposed bf16, wq/wk/wv column slices (kt-swizzled), full
Wo (kt-swizzled), bo pre-broadcast [128, 1024]; reassembles token rows.
"""

import sys

for p in ("/opt/trn_rl_repo", "/root/.axon_site/_ro/trn_rl_repo"):
    if p not in sys.path:
        sys.path.insert(0, p)

import numpy as np
import ml_dtypes

import bass_rust
import concourse.bass as bass
import concourse.mybir as mybir
from concourse.bass_utils import run_bass_kernel_spmd
from concourse.masks import make_identity
from concourse.tile import TileContext

B, S, D = 2, 2048, 1024
H, DH = 16, 64
T = B * S              # 4096 tokens
NC = 8                 # cores
HG = D // NC           # 128 qkv dims per core (2 heads)
KT_D = D // 128        # 8 contraction tiles over d_model
NQC = S // 512         # 4 q-chunks per batch
INV_SCALE = 1.0 / float(np.sqrt(DH))  # 1/8
F32 = mybir.dt.float32
F32R = mybir.dt.float32r
BF16 = mybir.dt.bfloat16
BFNP = ml_dtypes.bfloat16


def _split_waits(nc, max_waits=1):
    """This walrus build accepts one sync-wait per instruction; Tile sometimes
    emits more. Split extras into preceding NoOps on the same engine."""
    n = 0
    for f in nc.m.functions:
        for bb in f.blocks:
            out = []
            for inst in bb.instructions:
                si = getattr(inst, "sync_info", None)
                if si is not None and si.on_wait and len(si.on_wait) > max_waits:
                    waits = list(si.on_wait)
                    head, rest = waits[:-max_waits], waits[-max_waits:]
                    k = 0
                    while head:
                        chunk, head = head[:max_waits], head[max_waits:]
                        out.append(mybir.InstNoOp(
                            name=f"{inst.name}-wsplit-{k}", ins=[], outs=[],
                            engine=inst.engine,
                            sync_info=bass_rust.SyncInfo(on_wait=chunk, on_update=[]),
                        ))
                        k += 1
                    si.on_wait = rest
                    n += 1
                out.append(inst)
            bb.instructions = out
    return n


def build_module():
    nc = bass.Bass()

    xT = nc.dram_tensor("xT", [D, T], BF16, kind="ExternalInput")
    # weights host-preswizzled as [128, kt*n]: row p holds W[kt*128+p, n]
    wq = nc.dram_tensor("wq", [128, KT_D * HG], BF16, kind="ExternalInput")
    wk = nc.dram_tensor("wk", [128, KT_D * HG], BF16, kind="ExternalInput")
    wv = nc.dram_tensor("wv", [128, KT_D * HG], BF16, kind="ExternalInput")
    wo = nc.dram_tensor("wo", [128, KT_D * D], BF16, kind="ExternalInput")
    bq = nc.dram_tensor("bq", [HG, 1], F32, kind="ExternalInput")
    bk = nc.dram_tensor("bk", [HG, 1], F32, kind="ExternalInput")
    bv = nc.dram_tensor("bv", [HG, 1], F32, kind="ExternalInput")
    bo = nc.dram_tensor("bo", [128, D], F32, kind="ExternalInput")
    # y[(2b+hf)*128 + i, od] = out for token hf*1024 + rank*128 + i of batch b
    y = nc.dram_tensor("y", [B * 2 * 128, D], BF16, kind="ExternalOutput")

    # AllToAll staging per (batch, half): send block j = (my 128 dims,
    # 128 tokens of rank j); receive block j = (rank j's 128 dims, my
    # 128 tokens of that half)
    HT = 128
    a2i = [[nc.dram_tensor(f"a2i{b}_{h}", [NC, HG, HT], BF16)
            for h in range(2)] for b in range(B)]
    a2o = [[nc.dram_tensor(f"a2o{b}_{h}", [NC, HG, HT], BF16)
            for h in range(2)] for b in range(B)]

    with TileContext(nc) as tc:
        with tc.tile_pool(name="persist", bufs=1) as pp:
            # qkv weights as [128, kt, 128]; full Wo as [128, kt, 1024].
            # wq first on sync so the very first matmul's stationary
            # operand arrives before the x tiles finish.
            w_sb = {}
            for name, dram, eng in (("wq", wq, nc.sync), ("wk", wk, nc.scalar),
                                    ("wv", wv, nc.gpsimd)):
                t = pp.tile([128, KT_D, HG], BF16, name=f"{name}_sb", tag=f"{name}_sb")
                eng.dma_start(out=t[:], in_=dram[:].rearrange("p (kt n) -> p kt n", kt=KT_D))
                w_sb[name] = t
            # wo_sb DMA emitted at D(0) start (needed only by F)
            wo_sb = pp.tile([128, KT_D, D], BF16, name="wo_sb", tag="wo_sb")
            b_sb = {}
            for name, dram in (("bq", bq), ("bk", bk), ("bv", bv)):
                t = pp.tile([HG, 1], F32, name=f"{name}_sb", tag=f"{name}_sb")
                nc.sync.dma_start(out=t[:], in_=dram[:])
                b_sb[name] = t
            bo_sb = pp.tile([128, D], F32, name="bo_sb", tag="bo_sb")
            nc.sync.dma_start(out=bo_sb[:], in_=bo[:])

            # identity built in f32 (gpsimd memset can't write bf16 reliably),
            # then DVE-copied (rounds) into the bf16 tile matmul needs
            ident_f = pp.tile([128, 128], F32, name="ident_f", tag="ident_f")
            make_identity(nc, ident_f[:])
            ident = pp.tile([128, 128], BF16, name="ident", tag="ident")
            nc.vector.tensor_copy(ident[:], ident_f[:])
            # multiplicative causal mask for a diagonal 128x128 tile in
            # scores^T: tri01[r, c] = 1 where r <= c (k <= q), else 0
            tri_f = pp.tile([128, 128], F32, name="tri_f", tag="tri_f")
            nc.gpsimd.memset(tri_f[:], 1.0)
            nc.gpsimd.affine_select(
                out=tri_f[:], in_=tri_f[:],
                compare_op=mybir.AluOpType.is_ge, fill=0.0,
                base=0, pattern=[[1, 128]], channel_multiplier=-1,
            )
            tri01 = pp.tile([128, 128], BF16, name="tri01", tag="tri01")
            nc.vector.tensor_copy(tri01[:], tri_f[:])
            # ones row for the r-broadcast outer product (f32r, full speed)
            ones_r = pp.tile([65, 128], F32R, name="ones_r", tag="ones_r")
            of = pp.tile([65, 128], F32, name="of", tag="of")
            nc.vector.memset(of[:], 1.0)
            nc.vector.tensor_copy(ones_r[:], of[:])
            ones128 = pp.tile([128, 64], F32, name="ones128", tag="ones128")
            nc.vector.memset(ones128[:], 1.0)

            qkvT = {}
            for name in ("qT", "kT", "vT"):
                qkvT[name] = [pp.tile([128, S], BF16, name=f"{name}{b}", tag=f"{name}{b}")
                              for b in range(B)]

            vaug = pp.tile([128, B * 2, S // 128, DH + 1], BF16, name="vaug", tag="vaug")
            nc.vector.tensor_copy(vaug[:, :, :, DH:DH + 1], ones128[:, :])
            # [65 used partitions, pair, q]; row 64 = l
            ctxu = pp.tile([128, B * 2, S], F32, name="ctxu", tag="ctxu")

            # ---------------- stage B(b0): N=1024 matmuls, 3-PSUM ----------------
            xt0 = {}  # (tq) -> list of 8 [128,1024] tiles, batch 0
            with (
                tc.tile_pool(name="xt0_pool", bufs=16) as xt0_pool,
                tc.tile_pool(name="psB0", bufs=2, space="PSUM") as psB0_pool,
                tc.tile_pool(name="psT0", bufs=2, space="PSUM") as psT0_pool,
            ):
                dma_engs = (nc.sync, nc.scalar, nc.gpsimd)
                for tq in range(2):
                    t0 = tq * 1024
                    xts = []
                    for kt in range(KT_D):
                        xt = xt0_pool.tile([128, 1024], BF16, name="xt0", tag="xt0")
                        dma_engs[kt % 3].dma_start(
                            out=xt[:], in_=xT[kt * 128:(kt + 1) * 128, t0:t0 + 1024])
                        xts.append(xt)
                    xt0[tq] = xts
                    for c2 in range(2):
                        ps = [psB0_pool.tile([128, 512], F32, name=f"psB0{i}",
                                             tag=f"psB0{i}") for i in range(3)]
                        for kt in range(KT_D):
                            for pi, wname in enumerate(("wq", "wk", "wv")):
                                nc.tensor.matmul(
                                    ps[pi][:], w_sb[wname][:, kt, :],
                                    xts[kt][:, c2 * 512:(c2 + 1) * 512],
                                    start=(kt == 0), stop=(kt == KT_D - 1))
                        for pi, (dname, bname) in enumerate(
                                (("qT", "bq"), ("kT", "bk"), ("vT", "bv"))):
                            nc.vector.tensor_scalar_add(
                                out=qkvT[dname][0][:, t0 + c2 * 512:
                                                   t0 + (c2 + 1) * 512],
                                in0=ps[pi][:], scalar1=b_sb[bname][:, 0:1])
                # ---- stage C(b0) ----
                for h in range(2):
                    for g in range(2):
                        pst = psT0_pool.tile([128, 512], BF16, name="pst0", tag="pst0")
                        for j in range(8):
                            kt = g * 8 + j
                            nc.tensor.transpose(
                                out=pst[:, j * DH:(j + 1) * DH],
                                in_=qkvT["vT"][0][h * DH:(h + 1) * DH,
                                                  kt * 128:(kt + 1) * 128],
                                identity=ident[h * DH:(h + 1) * DH,
                                               h * DH:(h + 1) * DH])
                        nc.vector.tensor_copy(vaug[:, h, g * 8:(g + 1) * 8, 0:DH], pst[:])

            # ------- stages D/E/F -------
            with (
                tc.tile_pool(name="psS", bufs=2, space="PSUM") as psS_pool,      # 4 banks
                tc.tile_pool(name="psC", bufs=1, space="PSUM") as psC_pool,      # 2 banks
                tc.tile_pool(name="exp_pool", bufs=4) as exp_pool,
                tc.tile_pool(name="rpool", bufs=2) as rpool,
                tc.tile_pool(name="cn_pool", bufs=2) as cn_pool,
                tc.tile_pool(name="gx_pool", bufs=4) as gx_pool,
                tc.tile_pool(name="xt1_pool", bufs=16) as xt1_pool,
            ):
                def stage_E(b, hf, misc_pool):
                    """Normalize+stage tokens [hf*1024, +1024) of batch b and
                    fire the A2A. Yields between PE units so the caller can
                    interleave into D iterations."""
                    pr0 = b * 2
                    q0 = hf * 1024
                    # r = 1/l = exp(-ln(l)), both heads, whole half-batch
                    ln_f = rpool.tile([65, 2, 1024], F32, name="ln_f", tag="ln_f")
                    nc.scalar.activation(
                        out=ln_f[64:65, :, :],
                        in_=ctxu[64:65, pr0:pr0 + 2, q0:q0 + 1024],
                        func=mybir.ActivationFunctionType.Ln)
                    r_t = rpool.tile([65, 2, 1024], F32R, name="r_t", tag="r_t")
                    nc.scalar.activation(
                        out=r_t[64:65, :, :], in_=ln_f[64:65, :, :],
                        func=mybir.ActivationFunctionType.Exp, scale=-1.0)
                    cn = cn_pool.tile([128, 1024], BF16, name="cn", tag="cn")
                    yield
                    for h in range(2):
                        for c2 in range(2):
                            bcst = misc_pool.tile([64, 512], F32, name="bc", tag="bc")
                            nc.tensor.matmul(
                                bcst[0:DH, :],
                                ones_r[64:65, 0:DH],
                                r_t[64:65, h, c2 * 512:(c2 + 1) * 512],
                                start=True, stop=True)
                            nc.vector.tensor_mul(
                                out=cn[h * DH:(h + 1) * DH, c2 * 512:(c2 + 1) * 512],
                                in0=ctxu[0:DH, pr0 + h, q0 + c2 * 512:q0 + (c2 + 1) * 512],
                                in1=bcst[0:DH, :])
                            yield
                    for j in range(NC):
                        nc.sync.dma_start(
                            out=a2i[b][hf][j, :, :],
                            in_=cn[:, j * HT:(j + 1) * HT])
                    nc.gpsimd.collective_compute(
                        "AllToAll",
                        mybir.AluOpType.bypass,
                        ins=[a2i[b][hf][:]],
                        outs=[a2o[b][hf][:]],
                        replica_groups=[list(range(NC))],
                    )

                def stage_D(b, misc_pool):
                    """Generator: yields after each (qc, kt) unit so filler
                    work can interleave into the PE stream."""
                    pr0, pr1 = b * 2, b * 2 + 1
                    qT0 = qkvT["qT"][b][0:DH, :]
                    kT0 = qkvT["kT"][b][0:DH, :]
                    qT1 = qkvT["qT"][b][DH:2 * DH, :]
                    kT1 = qkvT["kT"][b][DH:2 * DH, :]
                    e_gen = None
                    for qc in range(NQC):
                        q0 = qc * 512
                        n_kt = q0 // 128 + 4
                        ps_c0 = psC_pool.tile([128, 512], F32, name="ps_c0",
                                              tag="ps_c0")
                        ps_c1 = psC_pool.tile([128, 512], F32, name="ps_c1",
                                              tag="ps_c1")
                        for kt in range(n_kt):
                            off = max(0, kt * 128 - q0)
                            ps_s = psS_pool.tile([128, 1024], F32, name="ps_s",
                                                 tag="ps_s")
                            # h0 on PE rows 0-63, h1 on rows 64-127: disjoint
                            # row groups -> the two matmuls run concurrently
                            nc.tensor.matmul(
                                ps_s[:, off:512],
                                kT0[:, kt * 128:(kt + 1) * 128],
                                qT0[:, q0 + off:q0 + 512],
                                start=True, stop=True)
                            nc.tensor.matmul(
                                ps_s[:, 512 + off:1024],
                                kT1[:, kt * 128:(kt + 1) * 128],
                                qT1[:, q0 + off:q0 + 512],
                                start=True, stop=True)
                            ex = exp_pool.tile([128, 1024], BF16, name="ex", tag="ex")
                            nc.scalar.activation(
                                out=ex[:, off:1024], in_=ps_s[:, off:1024],
                                func=mybir.ActivationFunctionType.Exp,
                                scale=INV_SCALE)
                            if kt * 128 >= q0:
                                nc.vector.tensor_mul(
                                    out=ex[:, off:off + 128],
                                    in0=ex[:, off:off + 128], in1=tri01[:])
                                nc.vector.tensor_mul(
                                    out=ex[:, 512 + off:512 + off + 128],
                                    in0=ex[:, 512 + off:512 + off + 128], in1=tri01[:])
                            nc.tensor.matmul(
                                ps_c0[0:DH + 1, off:512],
                                vaug[:, pr0, kt, :],
                                ex[:, off:512],
                                start=(kt == 0), stop=(kt == n_kt - 1),
                                skip_group_check=True)
                            nc.tensor.matmul(
                                ps_c1[0:DH + 1, off:512],
                                vaug[:, pr1, kt, :],
                                ex[:, 512 + off:1024],
                                start=(kt == 0), stop=(kt == n_kt - 1),
                                skip_group_check=True)
                            yield
                            if e_gen is not None:
                                # spread the previous half's E units into this
                                # chunk's PE stream
                                if next(e_gen, "done") == "done":
                                    e_gen = None
                        nc.vector.tensor_copy(
                            ctxu[0:DH + 1, pr0, q0:q0 + 512], ps_c0[0:DH + 1, :])
                        nc.vector.tensor_copy(
                            ctxu[0:DH + 1, pr1, q0:q0 + 512], ps_c1[0:DH + 1, :])
                        yield
                        if qc == 1:
                            e_gen = stage_E(b, 0, misc_pool)
                            next(e_gen)
                    # final half: drain E inline (no D iterations left)
                    if e_gen is not None:
                        for _ in e_gen:
                            pass
                    for _ in stage_E(b, 1, misc_pool):
                        pass

                def b1_prefetch():
                    """Emit batch-1 x loads up front so the first filler
                    matmul never stalls the PE queue head."""
                    engs = (nc.sync, nc.gpsimd)
                    xts = {}
                    for tq in range(2):
                        t0 = S + tq * 1024
                        tiles = []
                        for kt in range(KT_D):
                            xt = xt1_pool.tile([128, 1024], BF16, name="xt1", tag="xt1")
                            eng = engs[kt % 2]
                            eng.dma_start(
                                out=xt[:], in_=xT[kt * 128:(kt + 1) * 128, t0:t0 + 1024])
                            tiles.append(xt)
                        xts[tq] = tiles
                    return xts

                def b1_units(xts):
                    """Generator of stage-B(b1) PE units: one projection
                    matmul per next(); emits the DVE bias-add after each
                    8-matmul chunk."""
                    with tc.tile_pool(name="psB1", bufs=1, space="PSUM") as psB1_pool:
                        for tq in range(2):
                            t0 = tq * 1024
                            for wname, dname, bname in (
                                    ("wq", "qT", "bq"), ("wk", "kT", "bk"),
                                    ("wv", "vT", "bv")):
                                for c2 in range(2):
                                    ps = psB1_pool.tile([128, 512], F32,
                                                        name="psB1", tag="psB1")
                                    for kt in range(KT_D):
                                        nc.tensor.matmul(
                                            ps[:],
                                            w_sb[wname][:, kt, :],
                                            xts[tq][kt][:, c2 * 512:(c2 + 1) * 512],
                                            start=(kt == 0), stop=(kt == KT_D - 1))
                                        yield
                                    nc.vector.tensor_scalar_add(
                                        out=qkvT[dname][1][:, t0 + c2 * 512:
                                                           t0 + (c2 + 1) * 512],
                                        in0=ps[:], scalar1=b_sb[bname][:, 0:1])

                def gx_load(b, hf):
                    gx = gx_pool.tile([128, KT_D, HT], BF16, name="gx", tag="gx")
                    nc.sync.dma_start(
                        out=gx[:], in_=a2o[b][hf][:].rearrange("kt p t -> p kt t"))
                    return gx

                # ---- D(b0) with B(b1) interleaved ----
                nc.gpsimd.dma_start(
                    out=wo_sb[:], in_=wo[:].rearrange("p (kt n) -> p kt n", kt=KT_D))
                with tc.tile_pool(name="miscA", bufs=1, space="PSUM") as miscA:
                    filler = b1_units(b1_prefetch())
                    for _ in stage_D(0, miscA):
                        for _ in range(3):
                            if next(filler, "done") == "done":
                                break
                    for _ in filler:  # drain any remainder
                        pass
                # ---- C(b1) burst ----
                with tc.tile_pool(name="psT1", bufs=1, space="PSUM") as psT1_pool:
                    for h in range(2):
                        for g in range(2):
                            pst = psT1_pool.tile([128, 512], BF16, name="pst1",
                                                 tag="pst1")
                            for j in range(8):
                                kt = g * 8 + j
                                nc.tensor.transpose(
                                    out=pst[:, j * DH:(j + 1) * DH],
                                    in_=qkvT["vT"][1][h * DH:(h + 1) * DH,
                                                      kt * 128:(kt + 1) * 128],
                                    identity=ident[h * DH:(h + 1) * DH,
                                                   h * DH:(h + 1) * DH])
                            nc.vector.tensor_copy(
                                vaug[:, 2 + h, g * 8:(g + 1) * 8, 0:DH], pst[:])
                # ---- D(b1); gx prefetches slotted in ----
                gx_tiles = {}
                gx_tiles[(0, 0)] = gx_load(0, 0)
                gx_tiles[(0, 1)] = gx_load(0, 1)
                with tc.tile_pool(name="miscB", bufs=2, space="PSUM") as miscB:
                    d1 = stage_D(1, miscB)
                    for i, _ in enumerate(d1):
                        if i == 30:
                            gx_tiles[(1, 0)] = gx_load(1, 0)
                gx_tiles[(1, 1)] = gx_load(1, 1)

            # ---- stage F: transposed output projection ----
            with (
                tc.tile_pool(name="psF", bufs=2, space="PSUM") as psF_pool,
                tc.tile_pool(name="yo2_pool", bufs=2) as yo2_pool,
            ):
                for b in range(B):
                    for hf in range(2):
                        gx = gx_tiles[(b, hf)]
                        ps_o = psF_pool.tile([128, D], F32, name="ps_o", tag="ps_o")
                        for kt in range(KT_D):
                            for c2 in range(2):
                                nc.tensor.matmul(
                                    ps_o[:, c2 * 512:(c2 + 1) * 512],
                                    gx[:, kt, :],
                                    wo_sb[:, kt, c2 * 512:(c2 + 1) * 512],
                                    start=(kt == 0), stop=(kt == KT_D - 1),
                                    skip_group_check=True)
                        yo = yo2_pool.tile([128, D], BF16, name="yo", tag="yo")
                        nc.vector.tensor_add(out=yo[:], in0=ps_o[:], in1=bo_sb[:])
                        r0 = (2 * b + hf) * 128
                        nc.sync.dma_start(out=y[r0:r0 + 128, :], in_=yo[:])

    _split_waits(nc)
    return nc


def _swz(w):
    """[D, n] -> preswizzled [128, KT_D*n]: row p = concat_kt W[kt*128+p, :]."""
    n = w.shape[1]
    return np.ascontiguousarray(
        w.reshape(KT_D, 128, n).transpose(1, 0, 2).reshape(128, KT_D * n)
        .astype(BFNP))


def kernel(x, mask, Wq, bq, Wk, bk, Wv, bv, Wo, bo, trace=False):
    x = np.asarray(x, dtype=np.float32).reshape(T, D)
    xT = np.ascontiguousarray(x.T.astype(BFNP))
    wo_full = _swz(np.asarray(Wo, np.float32))
    bo_b = np.ascontiguousarray(
        np.broadcast_to(np.asarray(bo, np.float32), (128, D)))
    in_maps = []
    for c in range(NC):
        sl = slice(c * HG, (c + 1) * HG)
        in_maps.append({
            "xT": xT,
            "wq": _swz(np.asarray(Wq, np.float32)[:, sl]),
            "wk": _swz(np.asarray(Wk, np.float32)[:, sl]),
            "wv": _swz(np.asarray(Wv, np.float32)[:, sl]),
            "wo": wo_full,
            "bq": np.ascontiguousarray(np.asarray(bq, np.float32)[sl].reshape(HG, 1)),
            "bk": np.ascontiguousarray(np.asarray(bk, np.float32)[sl].reshape(HG, 1)),
            "bv": np.ascontiguousarray(np.asarray(bv, np.float32)[sl].reshape(HG, 1)),
            "bo": bo_b,
        })
    nc = build_module()
    res = run_bass_kernel_spmd(nc, in_maps, core_ids=list(range(NC)), trace=trace)
    out = np.empty((B, S, D), dtype=np.float32)
    for c in range(NC):
        yc = np.asarray(res.results[c]["y"], dtype=np.float32)  # [512, 1024]
        for b in range(B):
            for hf in range(2):
                t0 = hf * 1024 + c * 128
                out[b, t0:t0 + 128, :] = yc[(2 * b + hf) * 128:
                                            (2 * b + hf + 1) * 128, :]
    if trace:
        kernel.last_results = res
    return out
